# revision 15
# baseline (speedup 1.0000x reference)
"""Trainium2 Bass kernel for nn_DigitConvolutionalModel.

Model: x(B,784) -> reshape 28x28 -> 3x3 valid cross-correlation (kernel is an
input) -> flatten 676 -> Linear(676,128)+ReLU -> Linear(128,10).

Strategy:
  * Fold the 3x3 conv into the first linear layer on the host: the conv is a
    linear map, so h = relu(x @ W1eff.T + b1) with W1eff (128, 784) built by
    scattering conv_w-weighted copies of w1 onto the 28x28 grid. The device
    kernel is then a plain 2-layer MLP over 784 features.
  * Pure data parallelism: batch 65536 split as 8192 rows per NeuronCore,
    weights replicated.
  * The binding resource is per-SDMA-engine byte throughput (~17.5 GB/s x
    16 engines ~ 280 GB/s per core with all 8 cores streaming, regardless
    of dtype or descriptor size >= 14 KB). The PE only eats fp16/bf16/fp8
    moving operands (fp8 quantization of x fails the accuracy gate:
    3.1e-2 vs 2e-2), so the head blocks ship as int8 - q = round(x/s),
    one global scale s = absmax/127 folded exactly into the fp16 layer-1
    weights - and are cast int8->fp16 on the scalar/vector engines'
    spare capacity, while the tail blocks ship as fp16 (x/s) so the
    post-stream critical path has no cast latency. This cuts the stream
    from 12.85 MB (~46 us) to 9.6 MB (~34 us). Measured end-to-end error
    1.42e-2 of scale (inputs are deterministic, fixed seed).
  * x is shipped packed per 1024-row DMA block with each partition's block
    data one contiguous HBM run: 112 descriptors of 14 KB per block (the
    per-descriptor sweet spot; 4 KB descriptors throttle at ~240 GB/s).
  * Weights load first on the SAME sync ring as x - per-engine FIFO within
    a ring guarantees they land before block 0 (a separate queue would
    round-robin against the bulk x stream and arrive ~20 us late, stalling
    the first matmul and, via DMA-lane sem reuse, later x descriptor
    generation). Biases ride in the weight tile as fp16 and are widened to
    fp32 by one DVE op. Output stores ride the scalar ring so a waiting
    store never head-of-line-blocks an x load.
  * relu+b1 runs on the scalar engine (ACTIVATE Relu with bias AP,
    ~1 us per 1024-block), the b2-add evacuation on DVE; layer-2 work for
    block b is emitted between block b+1's layer-1 matmuls so the PE FIFO
    never waits on the epilogue. Compute tracks the DMA closely enough
    that the PE never idles past the ~3.4 us HAM window.
"""

from contextlib import ExitStack

import numpy as np

B = 65536
H = W = 28
K = 3
CH = CW = 26
FEAT = H * W          # 784
HID = 128
OUT = 10
NCORES = 8
BC = B // NCORES      # 8192 rows per core

KC = 112              # contraction-chunk partition size
KCH = 7               # chunks: 7 * 112 = 784
NT = 512              # batch rows per matmul (one PSUM bank fp32)
WCOL = KCH * HID      # 896 w1t columns in the packed weight tile
# wpk columns: [w1t 0:896][w2t 896:906][b1 906][b2 907]
WTOT = WCOL + OUT + 2

VARIANT = "f16"

_NC_CACHE = {}


def _blocks(bc):
    # 1024-row blocks (14 KB descriptors); small final blocks so the
    # post-stream compute tail is short
    if bc == 8192:
        blocks = [1024] * 7 + [512, 512]
    else:
        blocks = [min(1024, bc - o) for o in range(0, bc, 1024)]
    assert sum(blocks) == bc
    return blocks


def _build_nc(bc, variant):
    from concourse import bacc
    import concourse.mybir as mybir
    import concourse.tile as tile

    f32 = mybir.dt.float32
    f16 = mybir.dt.float16
    blocks = _blocks(bc)

    nc = bacc.Bacc(
        "TRN2",
        target_bir_lowering=False,
        debug=False,
        enable_asserts=False,
        num_devices=NCORES,
    )
    # [112, 7*bc] with per-block column groups: block b at columns
    # [7*off_b, 7*(off_b+xb)), chunk-major inside the block so each
    # partition's block data is one contiguous HBM run
    i8 = mybir.dt.int8
    # int8 blocks: 1..5 (block 0 stays fp16 so the PE starts without cast
    # latency; the tail blocks stay fp16 so the post-stream critical path
    # has no cast either)
    i8set = set(range(1, 6)) if bc == 8192 else set()
    bc8 = sum(xb for b, xb in enumerate(blocks) if b in i8set)
    xT8 = (
        nc.dram_tensor("xT8", [KC, KCH * bc8], i8, kind="ExternalInput").ap()
        if bc8
        else None
    )
    xT = nc.dram_tensor("xT", [KC, KCH * (bc - bc8)], f16, kind="ExternalInput").ap()
    wpk = nc.dram_tensor("wpk", [HID, WTOT], f16, kind="ExternalInput").ap()
    outT = nc.dram_tensor("outT", [OUT, bc], f32, kind="ExternalOutput").ap()

    with ExitStack() as ctx:
        tc = ctx.enter_context(tile.TileContext(nc))
        wpool = ctx.enter_context(tc.tile_pool(name="w", bufs=1))
        xpool = ctx.enter_context(tc.tile_pool(name="x", bufs=max(1, len(blocks) - 4)))
        xqpool = ctx.enter_context(tc.tile_pool(name="xq", bufs=4))
        xfpool = ctx.enter_context(tc.tile_pool(name="xf", bufs=4))
        hpool = ctx.enter_context(tc.tile_pool(name="h", bufs=3))
        opool = ctx.enter_context(tc.tile_pool(name="o", bufs=3))
        p1pool = ctx.enter_context(tc.tile_pool(name="p1", bufs=2, space="PSUM"))
        p2pool = ctx.enter_context(tc.tile_pool(name="p2", bufs=2, space="PSUM"))

        # weights first on the sync ring: FIFO within the ring means their
        # descriptors drain before block 0's
        ws = wpool.tile([HID, WTOT], f16)
        nc.sync.dma_start(ws[:], wpk[:])
        w2s = ws[:, WCOL : WCOL + OUT]
        bs = wpool.tile([HID, 2], f32)
        nc.vector.tensor_copy(bs[:], ws[:, WCOL + OUT : WCOL + OUT + 2])
        b1s = bs[:, 0:1]
        b2s = bs[0:OUT, 1:2]

        # int8 blocks: DMA int8, cast to fp16 on DVE/ACT (their spare
        # capacity; DVE casts run ~3.9us vs ACT ~6.3us per 1024 block).
        # fp16 blocks load directly. DMAs issue in block order on the one
        # ring; per-partition block data is one contiguous run -> one
        # descriptor per partition.
        xq_map = {}
        xs_list = [None] * len(blocks)
        off8 = 0
        off = 0
        for blk, xb in enumerate(blocks):
            if blk in i8set:
                xq = xqpool.tile([KC, KCH, xb], i8, tag="xq", name=f"xq_{blk}")
                nc.sync.dma_start(xq[:], xT8[:, KCH * off8 : KCH * (off8 + xb)])
                xq_map[blk] = xq
                off8 += xb
            else:
                xs = xpool.tile([KC, KCH, xb], f16, tag="xs", name=f"xs_{blk}")
                nc.sync.dma_start(xs[:], xT[:, KCH * off : KCH * (off + xb)])
                xs_list[blk] = xs
                off += xb
        # casts up front so they never queue behind epilogue ops; DVE
        # takes the first and every other one (it is the faster caster)
        for n, blk in enumerate(sorted(i8set)):
            xb = blocks[blk]
            xf = xfpool.tile([KC, KCH, xb], f16, tag="xf", name=f"xf_{blk}")
            if n % 2 == 0:
                nc.vector.tensor_copy(xf[:], xq_map[blk][:])
            else:
                nc.scalar.copy(xf[:], xq_map[blk][:])
            xs_list[blk] = xf

        add = mybir.AluOpType.add
        relu = mybir.ActivationFunctionType.Relu

        offs = []
        off = 0
        for xb in blocks:
            offs.append(off)
            off += xb

        def emit_l1(blk):
            xb = blocks[blk]
            xs = xs_list[blk]
            p1 = p1pool.tile([HID, xb], f32, tag="p1", name=f"p1_{blk}")
            for t0 in range(0, xb, NT):
                nt = min(NT, xb - t0)
                for c in range(KCH):
                    nc.tensor.matmul(
                        p1[:, t0 : t0 + nt],
                        ws[0:KC, c * HID : (c + 1) * HID],
                        xs[:, c, t0 : t0 + nt],
                        start=(c == 0),
                        stop=(c == KCH - 1),
                    )
            return p1

        def emit_l2(blk, p1):
            xb = blocks[blk]
            hs = hpool.tile([HID, xb], f16, tag="hs", name=f"hs_{blk}")
            # relu + b1 on the scalar engine: out = relu(p1 + b1)
            nc.scalar.activation(hs[:], p1[:], relu, bias=b1s)
            p2 = p2pool.tile([OUT, xb], f32, tag="p2", name=f"p2_{blk}")
            for t0 in range(0, xb, NT):
                nt = min(NT, xb - t0)
                nc.tensor.matmul(
                    p2[:, t0 : t0 + nt], w2s, hs[:, t0 : t0 + nt],
                    start=True, stop=True,
                )
            os_ = opool.tile([OUT, xb], f32, tag="os", name=f"os_{blk}")
            nc.vector.tensor_scalar_add(os_[:], p2[:], b2s)
            nc.scalar.dma_start(outT[:, offs[blk] : offs[blk] + xb], os_[:])

        prev = None
        for blk in range(len(blocks)):
            p1 = emit_l1(blk)
            if prev is not None:
                emit_l2(*prev)
            prev = (blk, p1)
        emit_l2(*prev)

    nc.compile()
    return nc


def get_nc(bc=BC, variant=VARIANT):
    key = (bc, variant)
    if key not in _NC_CACHE:
        _NC_CACHE[key] = _build_nc(bc, variant)
    return _NC_CACHE[key]


def _pack_xT(shard, blocks):
    """[bc, 784] row-major fp16 shard -> [112, 7*bc] per-block-contiguous."""
    parts = []
    off = 0
    for xb in blocks:
        sub = shard[off : off + xb]  # [xb, 784]
        # [xb, 7, 112] -> [112, 7, xb] -> [112, 7*xb]
        parts.append(sub.reshape(xb, KCH, KC).transpose(2, 1, 0).reshape(KC, KCH * xb))
        off += xb
    return np.ascontiguousarray(np.concatenate(parts, axis=1))


def _host_prep(x, conv_w, w1, b1, w2, b2, variant):
    """Fold conv into layer-1 weights and lay out per-core device inputs."""
    x = np.asarray(x, dtype=np.float32)
    conv_w = np.asarray(conv_w, dtype=np.float32)
    w1 = np.asarray(w1, dtype=np.float32)
    b1 = np.asarray(b1, dtype=np.float32)
    w2 = np.asarray(w2, dtype=np.float32)
    b2 = np.asarray(b2, dtype=np.float32)

    w1_img = w1.reshape(HID, CH, CW)
    w1eff = np.zeros((HID, H, W), dtype=np.float32)
    for di in range(K):
        for dj in range(K):
            w1eff[:, di : di + CH, dj : dj + CW] += conv_w[di, dj] * w1_img
    w1eff = w1eff.reshape(HID, FEAT)

    s = float(np.abs(x).max()) / 127.0
    w1eff = w1eff * s

    # w1t layout [112, 7*128]: chunk c partition p holds feature c*112+p
    w1t_host = (
        w1eff.T.reshape(KCH, KC, HID).transpose(1, 0, 2).reshape(KC, KCH * HID)
    )
    wpk_host = np.zeros((HID, WTOT), dtype=np.float32)
    wpk_host[0:KC, 0:WCOL] = w1t_host
    wpk_host[:, WCOL : WCOL + OUT] = w2.T
    wpk_host[:, WCOL + OUT] = b1
    wpk_host[0:OUT, WCOL + OUT + 1] = b2
    wpk_host = np.ascontiguousarray(wpk_host).astype(np.float16)

    blocks = _blocks(BC)
    i8set = set(range(1, 6)) if BC == 8192 else set()
    xs_f = (x * (1.0 / s)).astype(np.float16)
    xs_q = np.clip(np.round(x * (1.0 / s)), -127, 127).astype(np.int8)
    blk_off = []
    off = 0
    for xb in blocks:
        blk_off.append(off)
        off += xb
    f_blocks = [xb for b, xb in enumerate(blocks) if b not in i8set]
    q_blocks = [xb for b, xb in enumerate(blocks) if b in i8set]
    in_maps = []
    for c in range(NCORES):
        base = c * BC
        shard_f = np.concatenate(
            [
                xs_f[base + blk_off[b] : base + blk_off[b] + xb]
                for b, xb in enumerate(blocks)
                if b not in i8set
            ]
        )
        m = {"wpk": wpk_host, "xT": _pack_xT(shard_f, f_blocks)}
        if q_blocks:
            shard_q = np.concatenate(
                [
                    xs_q[base + blk_off[b] : base + blk_off[b] + xb]
                    for b, xb in enumerate(blocks)
                    if b in i8set
                ]
            )
            m["xT8"] = _pack_xT(shard_q, q_blocks)
        in_maps.append(m)
    return in_maps


def run(x, conv_w, w1, b1, w2, b2, trace=False, variant=VARIANT):
    from concourse.bass_utils import run_bass_kernel_spmd

    in_maps = _host_prep(x, conv_w, w1, b1, w2, b2, variant)
    nc = get_nc(BC, variant)
    res = run_bass_kernel_spmd(nc, in_maps, list(range(NCORES)), trace=trace)
    outT = np.concatenate([r["outT"] for r in res.results], axis=1)  # [10, B]
    return np.ascontiguousarray(outT.T), res


def kernel(x, conv_w, w1, b1, w2, b2):
    out, _ = run(x, conv_w, w1, b1, w2, b2)
    return out
